# revision 26
# baseline (speedup 1.0000x reference)
"""Fused transformer block (attention + SwiGLU MLP, RMS norms) on 8 TRN2 NeuronCores.

Sharding: tensor-parallel attention over heads (2 heads/core, w_qkv column-split,
w_o row-split, attn_bias head-split) followed by two token-sliced
ReduceScatters of the o_proj partials (rank r owns tokens
{h*1024 + r*128 .. +128} for h in {0,1}), then fully data-parallel MLP: every
core runs the complete SwiGLU on its 256 tokens with the full (unsharded)
gate/up/down weights, so both RMS norms and the second residual are core-local
and no second collective is needed. Phase 3 runs as two independent 128-token
half-pipelines (norm1 -> gate/up/silu -> down -> norm2 -> out), so the second
ReduceScatter overlaps the first half's MLP. The host gathers the 8 slices.

Attention softmax: exp(qk/8) on the scalar engine (q pre-scaled on host)
multiplied by host-precomputed exp(bias) on DVE; the softmax denominator rides
along as a ones-column in the PV matmul and is inverted with the fast
approximate DVE reciprocal.

Host-side prep: activations/weights pre-transposed and pre-cast (bf16,
q-columns of w_qkv pre-scaled by 1/sqrt(HD)).
"""

import sys

sys.path.insert(0, "/opt/trn_rl_repo")

import numpy as np
import ml_dtypes

import concourse.bass as bass
import concourse.mybir as mybir
import concourse.tile as tile
from concourse import bacc
from concourse.bass_utils import run_bass_kernel_spmd

P = 128
S = 2048
HID = 1024
NH = 16
HD = 64
INTER = 2816
EPS = 1e-5
N_CORES = 8
HPC = NH // N_CORES          # heads per core = 2
QC = 512                     # attention q-chunk
NQC = S // QC                # 4
KB = S // P                  # 16 k-blocks
KT = HID // P                # 8 hid contraction tiles
GKT = INTER // P             # 22 intermediate tiles
TLOC = S // N_CORES          # 256 tokens owned per core
TH = TLOC // 2               # 128 tokens per half-pipeline
F32 = mybir.dt.float32
BF16 = mybir.dt.bfloat16
FP8 = mybir.dt.float8e4

_cache = {}

USE_FAST_RECIP = True


def _recip(nc, out, in_):
    if USE_FAST_RECIP:
        nc.vector.reciprocal_approx_fast(out=out[:], in_=in_[:])
    else:
        nc.vector.reciprocal(out[:], in_[:])


def _build():
    nc = bacc.Bacc("TRN2", target_bir_lowering=False, debug=False,
                   num_devices=N_CORES)
    xbc = nc.dram_tensor("xbc", [HID, S], BF16, kind="ExternalInput").ap()
    xloc = nc.dram_tensor("xloc", [HID, TLOC], F32, kind="ExternalInput").ap()
    cs2 = nc.dram_tensor("cs2", [P, S], BF16, kind="ExternalInput").ap()
    sn2 = nc.dram_tensor("sn2", [P, S], BF16, kind="ExternalInput").ap()
    r2t = nc.dram_tensor("r2t", [P, P], BF16, kind="ExternalInput").ap()
    idb = nc.dram_tensor("idb", [P, P], BF16, kind="ExternalInput").ap()
    wqkv = nc.dram_tensor("wqkv", [HID, 3 * P], BF16, kind="ExternalInput").ap()
    wo = nc.dram_tensor("wo", [P, HID], BF16, kind="ExternalInput").ap()
    biasq = nc.dram_tensor("biasq", [HPC, S, S], BF16,
                           kind="ExternalInput").ap()
    wgu = nc.dram_tensor("wgu", [HID, 2 * INTER], BF16, kind="ExternalInput").ap()
    wdn = nc.dram_tensor("wdn", [INTER, HID], BF16, kind="ExternalInput").ap()
    outT = nc.dram_tensor("outT", [HID, TLOC], F32, kind="ExternalOutput").ap()

    with tile.TileContext(nc) as tc:
        _body(nc, tc, xbc, xloc, cs2, sn2, r2t, idb, wqkv, wo, biasq,
              wgu, wdn, outT)
    nc.compile()
    return nc


def _body(nc, tc, xbc, xloc, cs2, sn2, r2t, idb, wqkv, wo, biasq,
          wgu, wdn, outT):
    AF = mybir.ActivationFunctionType
    with tc.tile_pool(name="const", bufs=1) as const, \
         tc.tile_pool(name="dram1", bufs=1, space="DRAM") as dram1:
        o1c = [dram1.tile([N_CORES, HID, TH], BF16, tag=f"o1c{h}",
                          name=f"o1c{h}") for h in range(2)]
        o1sc = [dram1.tile([HID, TH], BF16, tag=f"o1sc{h}", name=f"o1sc{h}")
                for h in range(2)]
        wmark = dram1.tile([1, P], BF16, tag="wmark")

        # ---- full-kernel resident tensors ----
        wgu_sb = const.tile([P, KT, 2 * INTER], BF16, tag="wgu")   # 11.5 MB
        wdn_sb = const.tile([P, GKT, HID], BF16, tag="wdn")        # 5.8 MB
        xloc_sb = const.tile([P, KT, TLOC], F32, tag="xloc")       # 1 MB
        wo_sb = const.tile([P, HID], BF16, tag="wo")
        qT = const.tile([P, S], BF16, tag="qT")
        kTt = const.tile([P, S], BF16, tag="kT")
        vaug = const.tile([P, KB, 2 * (HD + 1)], BF16, tag="vaug")
        idb_sb = const.tile([P, P], BF16, tag="idb")
        onesb = const.tile([P, 1], BF16, tag="onesb")
        onesr = const.tile([1, P], F32, tag="onesr")
        misc = const.tile([P, 2], F32, tag="misc")
        sel0 = const.tile([1, P], F32, tag="sel0")
        sel1 = const.tile([1, P], F32, tag="sel1")
        eps_sb = misc[0:1, 0:1]
        nc.gpsimd.memset(eps_sb, EPS)
        nc.gpsimd.memset(onesb[:], 1.0)
        nc.gpsimd.memset(onesr[:], 1.0)
        nc.gpsimd.memset(sel0[:], 0.0)
        nc.gpsimd.memset(sel0[0:1, 0:HD], 1.0)
        nc.gpsimd.memset(sel1[:], 0.0)
        nc.gpsimd.memset(sel1[0:1, HD:P], 1.0)
        nc.gpsimd.memset(vaug[:, :, HD], 1.0)
        nc.gpsimd.memset(vaug[:, :, 2 * HD + 1], 1.0)

        # ============ phase 1: qkv projection, rope, v-transpose ============
        with tc.tile_pool(name="ph1", bufs=1) as ph1, \
             tc.tile_pool(name="xq_p", bufs=2) as xqp, \
             tc.tile_pool(name="wk_q", bufs=2) as wkq, \
             tc.tile_pool(name="ps_q", bufs=3, space="PSUM") as psq:
            # latency-critical loads on the sync queue, in need order
            wqkv_sb = ph1.tile([P, KT, 3 * P], BF16, tag="wqkv")
            nc.sync.dma_start(wqkv_sb[:], wqkv.rearrange("(t p) m -> p t m", p=P))
            cs_sb = ph1.tile([P, S], BF16, tag="cs")
            sn_sb = ph1.tile([P, S], BF16, tag="sn")
            r2b = ph1.tile([P, P], BF16, tag="r2b")

            xr = xbc.rearrange("(t p) s -> p t s", p=P)
            vT_bf = ph1.tile([P, S], BF16, tag="vT")
            for n in range(NQC):
                sl = slice(n * QC, (n + 1) * QC)
                xch = xqp.tile([P, KT, QC], BF16, tag="xch")
                nc.sync.dma_start(xch[:], xr[:, :, sl])
                if n == 0:
                    nc.sync.dma_start(cs_sb[:], cs2[:])
                    nc.sync.dma_start(sn_sb[:], sn2[:])
                    nc.sync.dma_start(r2b[:], r2t[:])
                    nc.sync.dma_start(idb_sb[:], idb[:])
                for part in range(3):                # q, k, v column blocks
                    ps = psq.tile([P, QC], F32, tag="mm")
                    for kt in range(KT):
                        nc.tensor.matmul(
                            ps[:],
                            lhsT=wqkv_sb[:, kt, part * P:(part + 1) * P],
                            rhs=xch[:, kt, :],
                            start=(kt == 0), stop=(kt == KT - 1),
                        )
                    if part == 0:
                        nc.scalar.copy(qT[:, sl], ps[:])
                    elif part == 1:
                        nc.scalar.copy(kTt[:, sl], ps[:])
                    else:
                        nc.vector.tensor_copy(vT_bf[:, sl], ps[:])

            # bulk prefetch on the Activation DGE queue, gated behind a
            # marker DMA that depends on the last qkv copy so the transfers
            # don't steal fabric bandwidth from the latency-critical stream
            nc.scalar.dma_start(wmark[:], qT[0:1, 0:P])
            nc.scalar.dma_start(wo_sb[:], wo[:])
            nc.scalar.dma_start(xloc_sb[:], xloc.rearrange("(t p) j -> p t j", p=P))
            nc.scalar.dma_start(wgu_sb[:], wgu.rearrange("(t p) m -> p t m", p=P))
            nc.scalar.dma_start(wdn_sb[:], wdn.rearrange("(g p) m -> p g m", p=P))

            # ---- RoPE on k then q:  t <- t*cos + (R2 @ t)*sin ----
            for t_sb in (kTt, qT):
                for n in range(NQC):
                    sl = slice(n * QC, (n + 1) * QC)
                    psr = psq.tile([P, QC], F32, tag="mm")
                    nc.tensor.matmul(psr[:], lhsT=r2b[:],
                                     rhs=t_sb[:, sl], start=True, stop=True)
                    m1 = wkq.tile([P, QC], BF16, tag="w512")
                    m2 = wkq.tile([P, QC], BF16, tag="w512b")
                    nc.vector.tensor_mul(out=m1[:], in0=t_sb[:, sl], in1=cs_sb[:, sl])
                    nc.vector.tensor_mul(out=m2[:], in0=psr[:], in1=sn_sb[:, sl])
                    nc.vector.tensor_add(out=t_sb[:, sl], in0=m1[:], in1=m2[:])

            # ---- v_aug: transpose v^T into [k, (v_h | 1)] blocks ----
            with tc.tile_pool(name="ps_t", bufs=2, space="PSUM") as pst:
                for kb in range(KB):
                    pt = pst.tile([P, P], BF16, tag="tr")
                    nc.tensor.transpose(pt[:], vT_bf[:, kb * P:(kb + 1) * P],
                                        idb_sb[:])
                    nc.vector.tensor_copy(vaug[:, kb, 0:HD], pt[:, 0:HD])
                    nc.vector.tensor_copy(vaug[:, kb, HD + 1:2 * HD + 1],
                                          pt[:, HD:2 * HD])

        # ======================= phase 2: attention =======================
        with tc.tile_pool(name="ps_s", bufs=2, space="PSUM") as pss, \
             tc.tile_pool(name="ps_pv", bufs=2, space="PSUM") as pspv, \
             tc.tile_pool(name="ps_zb", bufs=1, space="PSUM") as pszb, \
             tc.tile_pool(name="ps_o", bufs=1, space="PSUM") as pso, \
             tc.tile_pool(name="pt_p", bufs=6) as ptp, \
             tc.tile_pool(name="eb_p", bufs=5) as ebp, \
             tc.tile_pool(name="wk_a", bufs=2) as wka:
            for n in range(NQC):
                qsl = slice(n * QC, (n + 1) * QC)
                ebs = []
                for h in range(HPC):
                    for kq in range(4):          # 512-row k slabs of the bias
                        eb = ebp.tile([P, 4, QC], BF16, tag="eb")
                        nc.sync.dma_start(
                            eb[:],
                            biasq[h, kq * 512:(kq + 1) * 512, qsl].rearrange(
                                "(t p) q -> p t q", p=P))
                        ebs.append(eb)
                # scores + exp, two heads packed in PE row halves; the PV
                # accumulation is interleaved so each p tile is consumed
                # right after its exp (keeps the pT ring shallow)
                aoT = wka.tile([P, QC], BF16, tag="ao")
                zcs = [wka.tile([1, QC], F32, tag="zc", name=f"zc{h}")
                       for h in range(HPC)]
                zrs = [wka.tile([1, QC], F32, tag="zr", name=f"zr{h}")
                       for h in range(HPC)]
                pvs = [pspv.tile([HD + 1, QC], F32, tag="pv", name=f"pv{h}")
                       for h in range(HPC)]
                for kbp in range(KB // 2):
                    ps0 = pss.tile([P, 2, QC], F32, tag="qk")
                    ps1 = pss.tile([P, 2, QC], F32, tag="qk")
                    for i in range(2):
                        kb = 2 * kbp + i
                        ksl = slice(kb * P, (kb + 1) * P)
                        nc.tensor.matmul(ps0[:, i, :], lhsT=kTt[0:HD, ksl],
                                         rhs=qT[0:HD, qsl],
                                         start=True, stop=True,
                                         tile_position=(0, 0))
                        nc.tensor.matmul(ps1[:, i, :], lhsT=kTt[HD:P, ksl],
                                         rhs=qT[HD:P, qsl],
                                         start=True, stop=True,
                                         tile_position=(HD, 0))
                    for h, psh in ((0, ps0), (1, ps1)):
                        pt = ptp.tile([P, 2, QC], BF16, tag="pT")
                        ebh = ebs[h * 4 + kbp // 2]
                        nc.scalar.activation(pt[:], psh[:], AF.Exp)
                        for i in range(2):
                            kb = 2 * kbp + i
                            nc.vector.tensor_mul(
                                out=pt[:, i, :], in0=pt[:, i, :],
                                in1=ebh[:, kb % 4, :])
                        a0 = h * (HD + 1)
                        for i in range(2):
                            kb = 2 * kbp + i
                            nc.tensor.matmul(
                                pvs[h][:],
                                lhsT=vaug[:, kb, a0:a0 + HD + 1],
                                rhs=pt[:, i, :],
                                start=(kb == 0), stop=(kb == KB - 1),
                            )
                zbb = pszb.tile([P, QC], F32, tag="zbb")
                sels = (sel0, sel1)
                for h in range(HPC):
                    nc.vector.tensor_copy(zcs[h][:], pvs[h][HD:HD + 1, :])
                    _recip(nc, zrs[h], zcs[h])
                    nc.tensor.matmul(zbb[:], lhsT=sels[h][:], rhs=zrs[h][:],
                                     start=(h == 0), stop=(h == HPC - 1))
                zb = wka.tile([P, QC], F32, tag="zb")
                nc.scalar.copy(zb[:], zbb[:])
                for h in range(HPC):
                    nc.vector.tensor_mul(out=aoT[h * HD:(h + 1) * HD, :],
                                         in0=pvs[h][0:HD, :],
                                         in1=zb[h * HD:(h + 1) * HD, :])
                # o_proj partial; qc n holds the 128-token blocks of ranks
                # (n%2)*4 .. (n%2)*4+3 of half n//2
                r0 = (n % 2) * 4
                for m in range(KT):
                    po = pso.tile([P, QC], F32, tag="o")
                    nc.tensor.matmul(po[:], lhsT=wo_sb[:, m * P:(m + 1) * P],
                                     rhs=aoT[:], start=True, stop=True)
                    ob = wka.tile([P, QC], BF16, tag="ob")
                    nc.vector.tensor_copy(ob[:], po[:])
                    nc.sync.dma_start(
                        o1c[n // 2][r0:r0 + 4, m * P:(m + 1) * P, :].rearrange(
                            "r p j -> p r j"),
                        ob.rearrange("p (r j) -> p r j", r=4))
                if n % 2 == 1:
                    nc.gpsimd.collective_compute(
                        "ReduceScatter", mybir.AluOpType.add,
                        replica_groups=[list(range(N_CORES))],
                        ins=[o1c[n // 2].opt()], outs=[o1sc[n // 2].opt()],
                    )

        # ==== phase 3: two half-pipelines of norm1, DP SwiGLU, norm2 ====
        with tc.tile_pool(name="mlp", bufs=1) as mlp, \
             tc.tile_pool(name="wk_m", bufs=3) as wkm, \
             tc.tile_pool(name="ps_g", bufs=2, space="PSUM") as psg, \
             tc.tile_pool(name="ps_d", bufs=2, space="PSUM") as psd, \
             tc.tile_pool(name="ps_n", bufs=2, space="PSUM") as psn:
            x1bc = mlp.tile([P, KT, TLOC], BF16, tag="x1bc")
            actT = mlp.tile([P, GKT, TLOC], BF16, tag="actT")
            outr = outT.rearrange("(t p) j -> p t j", p=P)

            def local_norm(hsl, recast, out_dram):
                ss = psn.tile([1, TH], F32, tag="ss")
                for t in range(KT):
                    sq = wkm.tile([P, TH], BF16, tag="sq")
                    nc.scalar.square(sq[:], xloc_sb[:, t, hsl])
                    nc.tensor.matmul(ss[:], lhsT=onesb[:], rhs=sq[:],
                                     start=(t == 0), stop=(t == KT - 1))
                srow = wkm.tile([1, TH], F32, tag="srow")
                nc.scalar.activation(srow[:], ss[:], AF.Sqrt,
                                     bias=eps_sb, scale=1.0 / HID)
                rrow = wkm.tile([1, TH], F32, tag="rrow")
                _recip(nc, rrow, srow)
                rbp = psn.tile([P, TH], F32, tag="rbp")
                nc.tensor.matmul(rbp[:], lhsT=onesr[:], rhs=rrow[:],
                                 start=True, stop=True)
                rb = wkm.tile([P, TH], F32, tag="rb")
                nc.scalar.copy(rb[:], rbp[:])
                for t in range(KT):
                    if recast:
                        nc.vector.tensor_mul(out=x1bc[:, t, hsl],
                                             in0=xloc_sb[:, t, hsl], in1=rb[:])
                    nc.vector.tensor_mul(out=xloc_sb[:, t, hsl],
                                         in0=xloc_sb[:, t, hsl], in1=rb[:])
                    if out_dram is not None:
                        nc.sync.dma_start(out_dram[:, t, hsl],
                                          xloc_sb[:, t, hsl])

            for hf in range(2):
                hsl = slice(hf * TH, (hf + 1) * TH)
                o1l = mlp.tile([P, KT, TH], BF16, tag=f"o1l{hf}")
                nc.sync.dma_start(o1l[:],
                                  o1sc[hf].rearrange("(t p) j -> p t j", p=P))
                # residual 1 + norm1 on this half's tokens
                for t in range(KT):
                    nc.vector.tensor_add(out=xloc_sb[:, t, hsl],
                                         in0=xloc_sb[:, t, hsl],
                                         in1=o1l[:, t, :])
                local_norm(hsl, True, None)

                # gate/up + silu
                for g in range(GKT):
                    pg = psg.tile([P, 2, TH], F32, tag="gu")
                    for kt in range(KT):
                        nc.tensor.matmul(pg[:, 0, :],
                                         lhsT=wgu_sb[:, kt, g * P:(g + 1) * P],
                                         rhs=x1bc[:, kt, hsl],
                                         start=(kt == 0), stop=(kt == KT - 1))
                    for kt in range(KT):
                        nc.tensor.matmul(
                            pg[:, 1, :],
                            lhsT=wgu_sb[:, kt,
                                        INTER + g * P:INTER + (g + 1) * P],
                            rhs=x1bc[:, kt, hsl],
                            start=(kt == 0), stop=(kt == KT - 1))
                    sil = wkm.tile([P, TH], BF16, tag="sil")
                    if _cache.get("sim_safe_silu"):
                        # CoreSim has no Silu; emulate as x*sigmoid(x)
                        sg = wkm.tile([P, TH], BF16, tag="sg")
                        nc.scalar.activation(sg[:], pg[:, 0, :], AF.Sigmoid)
                        nc.vector.tensor_mul(out=sil[:], in0=sg[:],
                                             in1=pg[:, 0, :])
                    else:
                        nc.scalar.activation(sil[:], pg[:, 0, :], AF.Silu)
                    nc.vector.tensor_mul(out=actT[:, g, hsl], in0=sil[:],
                                         in1=pg[:, 1, :])

                # down proj + residual 2
                for mp in range(KT // 2):
                    pd = psd.tile([P, 2, TH], F32, tag="d")
                    for i in range(2):
                        m = 2 * mp + i
                        for g in range(GKT):
                            nc.tensor.matmul(pd[:, i, :],
                                             lhsT=wdn_sb[:, g,
                                                         m * P:(m + 1) * P],
                                             rhs=actT[:, g, hsl],
                                             start=(g == 0),
                                             stop=(g == GKT - 1))
                    for i in range(2):
                        nc.vector.tensor_add(out=xloc_sb[:, 2 * mp + i, hsl],
                                             in0=xloc_sb[:, 2 * mp + i, hsl],
                                             in1=pd[:, i, :])

                local_norm(hsl, False, outr)


def _prep_inputs(cos, sin, hidden_states, attn_bias, w_qkv, w_o, w_gate_up, w_down):
    bf = ml_dtypes.bfloat16
    xT = np.ascontiguousarray(hidden_states.reshape(S, HID).T.astype(np.float32))
    xbc = xT.astype(bf)
    cosT = cos.T.astype(np.float32)
    sinT = sin.T.astype(np.float32)
    cs2 = np.ascontiguousarray(np.concatenate([cosT, cosT], axis=0)).astype(bf)
    sn2 = np.ascontiguousarray(np.concatenate([sinT, sinT], axis=0)).astype(bf)
    # rotate_half as a left-multiply in transposed layout: R2 = blockdiag(R, R)
    R = np.zeros((HD, HD), np.float32)
    H2 = HD // 2
    for i in range(H2):
        R[i, i + H2] = -1.0
        R[i + H2, i] = 1.0
    R2 = np.zeros((2 * HD, 2 * HD), np.float32)
    R2[:HD, :HD] = R
    R2[HD:, HD:] = R
    r2t = np.ascontiguousarray(R2.T).astype(bf)
    idb = np.eye(P, dtype=np.float32).astype(bf)
    wgu_b = np.ascontiguousarray(w_gate_up).astype(bf)
    wdn_b = np.ascontiguousarray(w_down).astype(bf)

    in_maps = []
    for c in range(N_CORES):
        hA = HPC * c
        qcols = w_qkv[:, hA * HD:(hA + HPC) * HD] * 0.125  # fold 1/sqrt(HD)
        kcols = w_qkv[:, (NH + hA) * HD:(NH + hA + HPC) * HD]
        vcols = w_qkv[:, (2 * NH + hA) * HD:(2 * NH + hA + HPC) * HD]
        wqkv_c = np.ascontiguousarray(
            np.concatenate([qcols, kcols, vcols], axis=1)).astype(bf)
        wo_c = np.ascontiguousarray(w_o[hA * HD:(hA + HPC) * HD, :]).astype(bf)
        bT = attn_bias[0, hA:hA + HPC].transpose(0, 2, 1)  # [h][k][q]
        bias_c = np.ascontiguousarray(np.exp(bT)).astype(bf)
        xloc_c = np.empty((HID, TLOC), np.float32)
        for hf in range(2):
            xloc_c[:, hf * TH:(hf + 1) * TH] = \
                xT[:, hf * 1024 + c * TH:hf * 1024 + (c + 1) * TH]
        in_maps.append({
            "xbc": xbc, "xloc": xloc_c, "cs2": cs2, "sn2": sn2, "r2t": r2t,
            "idb": idb, "wqkv": wqkv_c, "wo": wo_c,
            "biasq": bias_c, "wgu": wgu_b, "wdn": wdn_b,
        })
    return in_maps


def kernel(cos, sin, hidden_states, attn_bias, w_qkv, w_o, w_gate_up, w_down,
           **_ignored):
    args = [np.asarray(a, np.float32) for a in
            (cos, sin, hidden_states, attn_bias, w_qkv, w_o, w_gate_up, w_down)]
    if "nc" not in _cache:
        _cache["nc"] = _build()
    nc = _cache["nc"]
    in_maps = _prep_inputs(*args)
    res = run_bass_kernel_spmd(nc, in_maps, core_ids=list(range(N_CORES)))
    _cache["last_results"] = res
    full = np.empty((HID, S), np.float32)
    for c in range(N_CORES):
        o = np.asarray(res.results[c]["outT"])
        for hf in range(2):
            full[:, hf * 1024 + c * TH:hf * 1024 + (c + 1) * TH] = \
                o[:, hf * TH:(hf + 1) * TH]
    return np.ascontiguousarray(full.T).reshape(1, S, HID).astype(np.float32)


# revision 27
# speedup vs baseline: 1.0503x; 1.0503x over previous
"""Fused transformer block (attention + SwiGLU MLP, RMS norms) on 8 TRN2 NeuronCores.

Sharding: tensor-parallel attention over heads (2 heads/core, w_qkv column-split,
w_o row-split, attn_bias head-split) followed by two token-sliced
ReduceScatters of the o_proj partials (rank r owns tokens
{h*1024 + r*128 .. +128} for h in {0,1}), then fully data-parallel MLP: every
core runs the complete SwiGLU on its 256 tokens with the full (unsharded)
gate/up/down weights, so both RMS norms and the second residual are core-local
and no second collective is needed. Phase 3 runs as two independent 128-token
half-pipelines (norm1 -> gate/up/silu -> down -> norm2 -> out), so the second
ReduceScatter overlaps the first half's MLP. The host gathers the 8 slices.

Attention softmax: exp(qk/8) on the scalar engine (q pre-scaled on host)
multiplied by host-precomputed exp(bias) on DVE; the softmax denominator rides
along as a ones-column in the PV matmul and is inverted with the fast
approximate DVE reciprocal.

Host-side prep: activations/weights pre-transposed and pre-cast (bf16,
q-columns of w_qkv pre-scaled by 1/sqrt(HD)).
"""

import sys

sys.path.insert(0, "/opt/trn_rl_repo")

import numpy as np
import ml_dtypes

import concourse.bass as bass
import concourse.mybir as mybir
import concourse.tile as tile
from concourse import bacc
from concourse.bass_utils import run_bass_kernel_spmd

P = 128
S = 2048
HID = 1024
NH = 16
HD = 64
INTER = 2816
EPS = 1e-5
N_CORES = 8
HPC = NH // N_CORES          # heads per core = 2
QC = 512                     # attention q-chunk
NQC = S // QC                # 4
KB = S // P                  # 16 k-blocks
KT = HID // P                # 8 hid contraction tiles
GKT = INTER // P             # 22 intermediate tiles
TLOC = S // N_CORES          # 256 tokens owned per core
TH = TLOC // 2               # 128 tokens per half-pipeline
F32 = mybir.dt.float32
BF16 = mybir.dt.bfloat16
FP8 = mybir.dt.float8e4

_cache = {}

USE_FAST_RECIP = True


def _recip(nc, out, in_):
    if USE_FAST_RECIP:
        nc.vector.reciprocal_approx_fast(out=out[:], in_=in_[:])
    else:
        nc.vector.reciprocal(out[:], in_[:])


def _build():
    nc = bacc.Bacc("TRN2", target_bir_lowering=False, debug=False,
                   num_devices=N_CORES)
    xbc = nc.dram_tensor("xbc", [HID, S], BF16, kind="ExternalInput").ap()
    xloc = nc.dram_tensor("xloc", [HID, TLOC], F32, kind="ExternalInput").ap()
    cs2 = nc.dram_tensor("cs2", [P, S], BF16, kind="ExternalInput").ap()
    sn2 = nc.dram_tensor("sn2", [P, S], BF16, kind="ExternalInput").ap()
    r2t = nc.dram_tensor("r2t", [P, P], BF16, kind="ExternalInput").ap()
    idb = nc.dram_tensor("idb", [P, P], BF16, kind="ExternalInput").ap()
    wqkv = nc.dram_tensor("wqkv", [HID, 3 * P], BF16, kind="ExternalInput").ap()
    wo = nc.dram_tensor("wo", [P, HID], BF16, kind="ExternalInput").ap()
    biasq = nc.dram_tensor("biasq", [HPC, S, S], BF16,
                           kind="ExternalInput").ap()
    wgu = nc.dram_tensor("wgu", [HID, 2 * INTER], BF16, kind="ExternalInput").ap()
    wdn = nc.dram_tensor("wdn", [INTER, HID], BF16, kind="ExternalInput").ap()
    outT = nc.dram_tensor("outT", [HID, TLOC], F32, kind="ExternalOutput").ap()

    with tile.TileContext(nc) as tc:
        _body(nc, tc, xbc, xloc, cs2, sn2, r2t, idb, wqkv, wo, biasq,
              wgu, wdn, outT)
    nc.compile()
    return nc


def _body(nc, tc, xbc, xloc, cs2, sn2, r2t, idb, wqkv, wo, biasq,
          wgu, wdn, outT):
    AF = mybir.ActivationFunctionType
    with tc.tile_pool(name="const", bufs=1) as const, \
         tc.tile_pool(name="dram1", bufs=1, space="DRAM") as dram1:
        o1c = [dram1.tile([N_CORES, HID, TH], BF16, tag=f"o1c{h}",
                          name=f"o1c{h}") for h in range(2)]
        o1sc = [dram1.tile([HID, TH], BF16, tag=f"o1sc{h}", name=f"o1sc{h}")
                for h in range(2)]
        wmark = dram1.tile([1, P], BF16, tag="wmark")

        # ---- full-kernel resident tensors ----
        wgu_sb = const.tile([P, KT, 2 * INTER], BF16, tag="wgu")   # 11.5 MB
        wdn_sb = const.tile([P, GKT, HID], BF16, tag="wdn")        # 5.8 MB
        xloc_sb = const.tile([P, KT, TLOC], F32, tag="xloc")       # 1 MB
        wo_sb = const.tile([P, HID], BF16, tag="wo")
        qT = const.tile([P, S], BF16, tag="qT")
        kTt = const.tile([P, S], BF16, tag="kT")
        vaug = const.tile([P, KB, 2 * (HD + 1)], BF16, tag="vaug")
        idb_sb = const.tile([P, P], BF16, tag="idb")
        onesb = const.tile([P, 1], BF16, tag="onesb")
        onesr = const.tile([1, P], F32, tag="onesr")
        misc = const.tile([P, 2], F32, tag="misc")
        sel0 = const.tile([1, P], F32, tag="sel0")
        sel1 = const.tile([1, P], F32, tag="sel1")
        eps_sb = misc[0:1, 0:1]
        nc.gpsimd.memset(eps_sb, EPS)
        nc.gpsimd.memset(onesb[:], 1.0)
        nc.gpsimd.memset(onesr[:], 1.0)
        nc.gpsimd.memset(sel0[:], 0.0)
        nc.gpsimd.memset(sel0[0:1, 0:HD], 1.0)
        nc.gpsimd.memset(sel1[:], 0.0)
        nc.gpsimd.memset(sel1[0:1, HD:P], 1.0)
        nc.gpsimd.memset(vaug[:, :, HD], 1.0)
        nc.gpsimd.memset(vaug[:, :, 2 * HD + 1], 1.0)

        # ============ phase 1: qkv projection, rope, v-transpose ============
        with tc.tile_pool(name="ph1", bufs=1) as ph1, \
             tc.tile_pool(name="xq_p", bufs=2) as xqp, \
             tc.tile_pool(name="wk_q", bufs=2) as wkq, \
             tc.tile_pool(name="ps_q", bufs=3, space="PSUM") as psq:
            # latency-critical loads on the sync queue, in need order
            wqkv_sb = ph1.tile([P, KT, 3 * P], BF16, tag="wqkv")
            nc.sync.dma_start(wqkv_sb[:], wqkv.rearrange("(t p) m -> p t m", p=P))
            cs_sb = ph1.tile([P, S], BF16, tag="cs")
            sn_sb = ph1.tile([P, S], BF16, tag="sn")
            r2b = ph1.tile([P, P], BF16, tag="r2b")

            xr = xbc.rearrange("(t p) s -> p t s", p=P)
            vT_bf = ph1.tile([P, S], BF16, tag="vT")
            for n in range(NQC):
                sl = slice(n * QC, (n + 1) * QC)
                xch = xqp.tile([P, KT, QC], BF16, tag="xch")
                nc.sync.dma_start(xch[:], xr[:, :, sl])
                if n == 0:
                    nc.sync.dma_start(cs_sb[:], cs2[:])
                    nc.sync.dma_start(sn_sb[:], sn2[:])
                    nc.sync.dma_start(r2b[:], r2t[:])
                    nc.sync.dma_start(idb_sb[:], idb[:])
                for part in range(3):                # q, k, v column blocks
                    ps = psq.tile([P, QC], F32, tag="mm")
                    for kt in range(KT):
                        nc.tensor.matmul(
                            ps[:],
                            lhsT=wqkv_sb[:, kt, part * P:(part + 1) * P],
                            rhs=xch[:, kt, :],
                            start=(kt == 0), stop=(kt == KT - 1),
                        )
                    if part == 0:
                        nc.scalar.copy(qT[:, sl], ps[:])
                    elif part == 1:
                        nc.scalar.copy(kTt[:, sl], ps[:])
                    else:
                        nc.vector.tensor_copy(vT_bf[:, sl], ps[:])

            # bulk prefetch on the Activation DGE queue, gated behind a
            # marker DMA that depends on the last qkv copy so the transfers
            # don't steal fabric bandwidth from the latency-critical stream
            nc.scalar.dma_start(wmark[:], qT[0:1, 0:P])
            nc.scalar.dma_start(wo_sb[:], wo[:])
            nc.scalar.dma_start(xloc_sb[:], xloc.rearrange("(t p) j -> p t j", p=P))
            nc.scalar.dma_start(wgu_sb[:], wgu.rearrange("(t p) m -> p t m", p=P))
            nc.scalar.dma_start(wdn_sb[:], wdn.rearrange("(g p) m -> p g m", p=P))

            # ---- RoPE on k then q:  t <- t*cos + (R2 @ t)*sin ----
            for t_sb in (kTt, qT):
                for n in range(NQC):
                    sl = slice(n * QC, (n + 1) * QC)
                    psr = psq.tile([P, QC], F32, tag="mm")
                    nc.tensor.matmul(psr[:], lhsT=r2b[:],
                                     rhs=t_sb[:, sl], start=True, stop=True)
                    m1 = wkq.tile([P, QC], BF16, tag="w512")
                    m2 = wkq.tile([P, QC], BF16, tag="w512b")
                    nc.vector.tensor_mul(out=m1[:], in0=t_sb[:, sl], in1=cs_sb[:, sl])
                    nc.vector.tensor_mul(out=m2[:], in0=psr[:], in1=sn_sb[:, sl])
                    nc.vector.tensor_add(out=t_sb[:, sl], in0=m1[:], in1=m2[:])

            # ---- v_aug: transpose v^T into [k, (v_h | 1)] blocks ----
            with tc.tile_pool(name="ps_t", bufs=2, space="PSUM") as pst:
                for kb in range(KB):
                    pt = pst.tile([P, P], BF16, tag="tr")
                    nc.tensor.transpose(pt[:], vT_bf[:, kb * P:(kb + 1) * P],
                                        idb_sb[:])
                    nc.vector.tensor_copy(vaug[:, kb, 0:HD], pt[:, 0:HD])
                    nc.vector.tensor_copy(vaug[:, kb, HD + 1:2 * HD + 1],
                                          pt[:, HD:2 * HD])

        # ======================= phase 2: attention =======================
        with tc.tile_pool(name="ps_s", bufs=2, space="PSUM") as pss, \
             tc.tile_pool(name="ps_pv", bufs=2, space="PSUM") as pspv, \
             tc.tile_pool(name="ps_zb", bufs=1, space="PSUM") as pszb, \
             tc.tile_pool(name="ps_o", bufs=1, space="PSUM") as pso, \
             tc.tile_pool(name="pt_p", bufs=6) as ptp, \
             tc.tile_pool(name="eb_p", bufs=5) as ebp, \
             tc.tile_pool(name="wk_a", bufs=2) as wka:
            for n in range(NQC):
                qsl = slice(n * QC, (n + 1) * QC)
                ebs = []
                for h in range(HPC):
                    for kq in range(4):          # 512-row k slabs of the bias
                        eb = ebp.tile([P, 4, QC], BF16, tag="eb")
                        nc.sync.dma_start(
                            eb[:],
                            biasq[h, kq * 512:(kq + 1) * 512, qsl].rearrange(
                                "(t p) q -> p t q", p=P))
                        ebs.append(eb)
                # scores + exp, two heads packed in PE row halves; the PV
                # accumulation is interleaved so each p tile is consumed
                # right after its exp (keeps the pT ring shallow)
                aoT = wka.tile([P, QC], BF16, tag="ao")
                zcs = [wka.tile([1, QC], F32, tag="zc", name=f"zc{h}")
                       for h in range(HPC)]
                zrs = [wka.tile([1, QC], F32, tag="zr", name=f"zr{h}")
                       for h in range(HPC)]
                pvs = [pspv.tile([HD + 1, QC], F32, tag="pv", name=f"pv{h}")
                       for h in range(HPC)]
                for kbp in range(KB // 2):
                    ps0 = pss.tile([P, 2, QC], F32, tag="qk")
                    ps1 = pss.tile([P, 2, QC], F32, tag="qk")
                    for i in range(2):
                        kb = 2 * kbp + i
                        ksl = slice(kb * P, (kb + 1) * P)
                        nc.tensor.matmul(ps0[:, i, :], lhsT=kTt[0:HD, ksl],
                                         rhs=qT[0:HD, qsl],
                                         start=True, stop=True,
                                         tile_position=(0, 0))
                        nc.tensor.matmul(ps1[:, i, :], lhsT=kTt[HD:P, ksl],
                                         rhs=qT[HD:P, qsl],
                                         start=True, stop=True,
                                         tile_position=(HD, 0))
                    for h, psh in ((0, ps0), (1, ps1)):
                        pt = ptp.tile([P, 2, QC], BF16, tag="pT")
                        ebh = ebs[h * 4 + kbp // 2]
                        nc.scalar.activation(pt[:], psh[:], AF.Exp)
                        for i in range(2):
                            kb = 2 * kbp + i
                            nc.vector.tensor_mul(
                                out=pt[:, i, :], in0=pt[:, i, :],
                                in1=ebh[:, kb % 4, :])
                        a0 = h * (HD + 1)
                        for i in range(2):
                            kb = 2 * kbp + i
                            nc.tensor.matmul(
                                pvs[h][:],
                                lhsT=vaug[:, kb, a0:a0 + HD + 1],
                                rhs=pt[:, i, :],
                                start=(kb == 0), stop=(kb == KB - 1),
                            )
                zbb = pszb.tile([P, QC], F32, tag="zbb")
                sels = (sel0, sel1)
                for h in range(HPC):
                    nc.vector.tensor_copy(zcs[h][:], pvs[h][HD:HD + 1, :])
                    _recip(nc, zrs[h], zcs[h])
                    nc.tensor.matmul(zbb[:], lhsT=sels[h][:], rhs=zrs[h][:],
                                     start=(h == 0), stop=(h == HPC - 1))
                zb = wka.tile([P, QC], F32, tag="zb")
                nc.scalar.copy(zb[:], zbb[:])
                for h in range(HPC):
                    nc.vector.tensor_mul(out=aoT[h * HD:(h + 1) * HD, :],
                                         in0=pvs[h][0:HD, :],
                                         in1=zb[h * HD:(h + 1) * HD, :])
                # o_proj partial; qc n holds the 128-token blocks of ranks
                # (n%2)*4 .. (n%2)*4+3 of half n//2
                r0 = (n % 2) * 4
                for m in range(KT):
                    po = pso.tile([P, QC], F32, tag="o")
                    nc.tensor.matmul(po[:], lhsT=wo_sb[:, m * P:(m + 1) * P],
                                     rhs=aoT[:], start=True, stop=True)
                    ob = wka.tile([P, QC], BF16, tag="ob")
                    nc.vector.tensor_copy(ob[:], po[:])
                    nc.scalar.dma_start(
                        o1c[n // 2][r0:r0 + 4, m * P:(m + 1) * P, :].rearrange(
                            "r p j -> p r j"),
                        ob.rearrange("p (r j) -> p r j", r=4))
                if n % 2 == 1:
                    nc.gpsimd.collective_compute(
                        "ReduceScatter", mybir.AluOpType.add,
                        replica_groups=[list(range(N_CORES))],
                        ins=[o1c[n // 2].opt()], outs=[o1sc[n // 2].opt()],
                    )

        # ==== phase 3: two half-pipelines of norm1, DP SwiGLU, norm2 ====
        with tc.tile_pool(name="mlp", bufs=1) as mlp, \
             tc.tile_pool(name="wk_m", bufs=3) as wkm, \
             tc.tile_pool(name="ps_g", bufs=2, space="PSUM") as psg, \
             tc.tile_pool(name="ps_d", bufs=2, space="PSUM") as psd, \
             tc.tile_pool(name="ps_n", bufs=2, space="PSUM") as psn:
            x1bc = mlp.tile([P, KT, TLOC], BF16, tag="x1bc")
            actT = mlp.tile([P, GKT, TLOC], BF16, tag="actT")
            outr = outT.rearrange("(t p) j -> p t j", p=P)

            def local_norm(hsl, recast, out_dram):
                ss = psn.tile([1, TH], F32, tag="ss")
                for t in range(KT):
                    sq = wkm.tile([P, TH], BF16, tag="sq")
                    nc.scalar.square(sq[:], xloc_sb[:, t, hsl])
                    nc.tensor.matmul(ss[:], lhsT=onesb[:], rhs=sq[:],
                                     start=(t == 0), stop=(t == KT - 1))
                srow = wkm.tile([1, TH], F32, tag="srow")
                nc.scalar.activation(srow[:], ss[:], AF.Sqrt,
                                     bias=eps_sb, scale=1.0 / HID)
                rrow = wkm.tile([1, TH], F32, tag="rrow")
                _recip(nc, rrow, srow)
                rbp = psn.tile([P, TH], F32, tag="rbp")
                nc.tensor.matmul(rbp[:], lhsT=onesr[:], rhs=rrow[:],
                                 start=True, stop=True)
                rb = wkm.tile([P, TH], F32, tag="rb")
                nc.scalar.copy(rb[:], rbp[:])
                for t in range(KT):
                    if recast:
                        nc.vector.tensor_mul(out=x1bc[:, t, hsl],
                                             in0=xloc_sb[:, t, hsl], in1=rb[:])
                    nc.vector.tensor_mul(out=xloc_sb[:, t, hsl],
                                         in0=xloc_sb[:, t, hsl], in1=rb[:])
                    if out_dram is not None:
                        nc.scalar.dma_start(out_dram[:, t, hsl],
                                            xloc_sb[:, t, hsl])

            for hf in range(2):
                hsl = slice(hf * TH, (hf + 1) * TH)
                o1l = mlp.tile([P, KT, TH], BF16, tag=f"o1l{hf}")
                nc.sync.dma_start(o1l[:],
                                  o1sc[hf].rearrange("(t p) j -> p t j", p=P))
                # residual 1 + norm1 on this half's tokens
                for t in range(KT):
                    nc.vector.tensor_add(out=xloc_sb[:, t, hsl],
                                         in0=xloc_sb[:, t, hsl],
                                         in1=o1l[:, t, :])
                local_norm(hsl, True, None)

                # gate/up + silu
                for g in range(GKT):
                    pg = psg.tile([P, 2, TH], F32, tag="gu")
                    for kt in range(KT):
                        nc.tensor.matmul(pg[:, 0, :],
                                         lhsT=wgu_sb[:, kt, g * P:(g + 1) * P],
                                         rhs=x1bc[:, kt, hsl],
                                         start=(kt == 0), stop=(kt == KT - 1))
                    for kt in range(KT):
                        nc.tensor.matmul(
                            pg[:, 1, :],
                            lhsT=wgu_sb[:, kt,
                                        INTER + g * P:INTER + (g + 1) * P],
                            rhs=x1bc[:, kt, hsl],
                            start=(kt == 0), stop=(kt == KT - 1))
                    sil = wkm.tile([P, TH], BF16, tag="sil")
                    if _cache.get("sim_safe_silu"):
                        # CoreSim has no Silu; emulate as x*sigmoid(x)
                        sg = wkm.tile([P, TH], BF16, tag="sg")
                        nc.scalar.activation(sg[:], pg[:, 0, :], AF.Sigmoid)
                        nc.vector.tensor_mul(out=sil[:], in0=sg[:],
                                             in1=pg[:, 0, :])
                    else:
                        nc.scalar.activation(sil[:], pg[:, 0, :], AF.Silu)
                    nc.vector.tensor_mul(out=actT[:, g, hsl], in0=sil[:],
                                         in1=pg[:, 1, :])

                # down proj + residual 2
                for mp in range(KT // 2):
                    pd = psd.tile([P, 2, TH], F32, tag="d")
                    for i in range(2):
                        m = 2 * mp + i
                        for g in range(GKT):
                            nc.tensor.matmul(pd[:, i, :],
                                             lhsT=wdn_sb[:, g,
                                                         m * P:(m + 1) * P],
                                             rhs=actT[:, g, hsl],
                                             start=(g == 0),
                                             stop=(g == GKT - 1))
                    for i in range(2):
                        nc.vector.tensor_add(out=xloc_sb[:, 2 * mp + i, hsl],
                                             in0=xloc_sb[:, 2 * mp + i, hsl],
                                             in1=pd[:, i, :])

                local_norm(hsl, False, outr)


def _prep_inputs(cos, sin, hidden_states, attn_bias, w_qkv, w_o, w_gate_up, w_down):
    bf = ml_dtypes.bfloat16
    xT = np.ascontiguousarray(hidden_states.reshape(S, HID).T.astype(np.float32))
    xbc = xT.astype(bf)
    cosT = cos.T.astype(np.float32)
    sinT = sin.T.astype(np.float32)
    cs2 = np.ascontiguousarray(np.concatenate([cosT, cosT], axis=0)).astype(bf)
    sn2 = np.ascontiguousarray(np.concatenate([sinT, sinT], axis=0)).astype(bf)
    # rotate_half as a left-multiply in transposed layout: R2 = blockdiag(R, R)
    R = np.zeros((HD, HD), np.float32)
    H2 = HD // 2
    for i in range(H2):
        R[i, i + H2] = -1.0
        R[i + H2, i] = 1.0
    R2 = np.zeros((2 * HD, 2 * HD), np.float32)
    R2[:HD, :HD] = R
    R2[HD:, HD:] = R
    r2t = np.ascontiguousarray(R2.T).astype(bf)
    idb = np.eye(P, dtype=np.float32).astype(bf)
    wgu_b = np.ascontiguousarray(w_gate_up).astype(bf)
    wdn_b = np.ascontiguousarray(w_down).astype(bf)

    in_maps = []
    for c in range(N_CORES):
        hA = HPC * c
        qcols = w_qkv[:, hA * HD:(hA + HPC) * HD] * 0.125  # fold 1/sqrt(HD)
        kcols = w_qkv[:, (NH + hA) * HD:(NH + hA + HPC) * HD]
        vcols = w_qkv[:, (2 * NH + hA) * HD:(2 * NH + hA + HPC) * HD]
        wqkv_c = np.ascontiguousarray(
            np.concatenate([qcols, kcols, vcols], axis=1)).astype(bf)
        wo_c = np.ascontiguousarray(w_o[hA * HD:(hA + HPC) * HD, :]).astype(bf)
        bT = attn_bias[0, hA:hA + HPC].transpose(0, 2, 1)  # [h][k][q]
        bias_c = np.ascontiguousarray(np.exp(bT)).astype(bf)
        xloc_c = np.empty((HID, TLOC), np.float32)
        for hf in range(2):
            xloc_c[:, hf * TH:(hf + 1) * TH] = \
                xT[:, hf * 1024 + c * TH:hf * 1024 + (c + 1) * TH]
        in_maps.append({
            "xbc": xbc, "xloc": xloc_c, "cs2": cs2, "sn2": sn2, "r2t": r2t,
            "idb": idb, "wqkv": wqkv_c, "wo": wo_c,
            "biasq": bias_c, "wgu": wgu_b, "wdn": wdn_b,
        })
    return in_maps


def kernel(cos, sin, hidden_states, attn_bias, w_qkv, w_o, w_gate_up, w_down,
           **_ignored):
    args = [np.asarray(a, np.float32) for a in
            (cos, sin, hidden_states, attn_bias, w_qkv, w_o, w_gate_up, w_down)]
    if "nc" not in _cache:
        _cache["nc"] = _build()
    nc = _cache["nc"]
    in_maps = _prep_inputs(*args)
    res = run_bass_kernel_spmd(nc, in_maps, core_ids=list(range(N_CORES)))
    _cache["last_results"] = res
    full = np.empty((HID, S), np.float32)
    for c in range(N_CORES):
        o = np.asarray(res.results[c]["outT"])
        for hf in range(2):
            full[:, hf * 1024 + c * TH:hf * 1024 + (c + 1) * TH] = \
                o[:, hf * TH:(hf + 1) * TH]
    return np.ascontiguousarray(full.T).reshape(1, S, HID).astype(np.float32)
